# revision 1
# baseline (speedup 1.0000x reference)
"""Trainium2 Bass kernel for nn_LinformerProjectionEntireOutImg.

Math: the reference's softmax is over a constant tensor -> uniform 1/64, so
the whole net collapses to a linear pipeline:
  T[b,q,i,j]  = sum_p cp[b, p*128+q, i, :] @ wc[p*128+q, :, j]   (q = n mod 128)
  S[b, r]     = T.reshape(B, 8192),  r = q*64 + i*8 + j
  P2[b,e]     = S @ E_proj.reshape(8192, 256)
  v[b,k]      = (P2[b,k]+P2[b,64+k]+P2[b,128+k]+P2[b,192+k])/64 + rel[k]
  out[b,o,i,j]= sum_m v[b, i*8+m] * w_next[o, m, j]
Sharding: core c owns capsule groups q in [16c, 16c+16) (== heads 4c..4c+4),
batch unsharded. Each core reads a disjoint 1/8 of current_pose/w_current and
1/8 of E_proj. The pipeline is linear, so each core emits its partial output
(core 0 carries the rel_embedd affine term) and the unshard is a sum.
The 1/64 is folded into E on the host; stage 1/2 run in bf16 (fp32 PSUM
accumulate), stage 3 in fp32.
"""

import os

import numpy as np

_STATE: dict = {}

B, OUT_N, POSE = 32, 64, 64
NCORES = 8

# p-chunk boundaries for the streamed stage-1 operand: small first chunks so
# the first matmul starts early, then large chunks to amortize DMA issue.
P_BOUNDS = [0, 2, 4, 10, 16, 22, 28, 34, 40, 46, 52, 58, 64]


def _build_nc():
    import concourse.mybir as mybir
    from concourse import bacc
    from concourse.tile import TileContext

    f32 = mybir.dt.float32
    bf16 = mybir.dt.bfloat16
    nc = bacc.Bacc()
    AW = nc.dram_tensor("aw_pack", [128, 64 * 384], bf16, kind="ExternalInput")
    E = nc.dram_tensor("e_pack", [128, 2048], bf16, kind="ExternalInput")
    REL = nc.dram_tensor("rel32", [32, 64], f32, kind="ExternalInput")
    WN = nc.dram_tensor("wn_pack", [8, 512], f32, kind="ExternalInput")
    IDT = nc.dram_tensor("ident32", [32, 32], f32, kind="ExternalInput")
    OUT = nc.dram_tensor("out", [2, 128, 512], f32, kind="ExternalOutput")

    with TileContext(nc) as tc:
        with (
            tc.tile_pool(name="apool", bufs=len(P_BOUNDS) - 1) as apool,
            tc.tile_pool(name="epool", bufs=1) as epool,
            tc.tile_pool(name="cpool", bufs=1) as cpool,
            tc.tile_pool(name="spool", bufs=1) as spool,
            tc.tile_pool(name="pp", bufs=1, space="PSUM") as pp,
            tc.tile_pool(name="pp3", bufs=2, space="PSUM") as pp3,
        ):
            # AW chunk DMAs, alternating between the two HWDGE queues.
            awts = []
            et = None
            relt = idt = wnt = None
            for ci in range(len(P_BOUNDS) - 1):
                w = (P_BOUNDS[ci + 1] - P_BOUNDS[ci]) * 384
                awt = apool.tile([128, w], bf16, tag="aw")
                eng = (nc.sync, nc.scalar)[ci % 2]
                eng.dma_start(
                    out=awt[:],
                    in_=AW[:, P_BOUNDS[ci] * 384 : P_BOUNDS[ci + 1] * 384],
                )
                awts.append(awt)
                if ci == 1:
                    et = epool.tile([128, 2048], bf16, tag="e")
                    nc.scalar.dma_start(out=et[:], in_=E[:])
                    relt = cpool.tile([32, 64], f32, tag="rel")
                    nc.sync.dma_start(out=relt[:], in_=REL[:])
                    idt = cpool.tile([32, 32], f32, tag="idt")
                    nc.sync.dma_start(out=idt[:], in_=IDT[:])
                    wnt = cpool.tile([8, 512], f32, tag="wn")
                    nc.sync.dma_start(out=wnt[:], in_=WN[:])

            # stage 1: T[(q,j),(i,b)] = sum_p Wblk_p.T @ A_p  (block-diag over q)
            # Two interleaved accumulation chains (even/odd p) in separate
            # PSUM banks so per-matmul ordering waits don't serialize the PE.
            o_ps0 = pp.tile([128, 256], f32, tag="o_ps0")
            o_ps1 = pp.tile([128, 256], f32, tag="o_ps1")
            for ci in range(len(P_BOUNDS) - 1):
                awt = awts[ci]
                for t in range(P_BOUNDS[ci + 1] - P_BOUNDS[ci]):
                    p = P_BOUNDS[ci] + t
                    tgt = o_ps0 if p % 2 == 0 else o_ps1
                    nc.tensor.matmul(
                        tgt[:],
                        awt[:, t * 384 + 256 : (t + 1) * 384],
                        awt[:, t * 384 : t * 384 + 256],
                        start=(p < 2),
                        stop=(p >= 62),
                    )
            o_half = spool.tile([128, 256], f32, tag="ohalf")
            nc.vector.tensor_copy(o_half[:], o_ps0[:])
            o_sb = spool.tile([128, 256], bf16, tag="osb")
            nc.vector.tensor_add(o_sb[:], o_half[:], o_ps1[:])

            # stage 2: P2[b,e] += O[:, i-cols].T @ (E_i/64)  (accumulate over i)
            p2_ps = pp.tile([32, 256], f32, tag="p2_ps")
            for i in range(8):
                nc.tensor.matmul(
                    p2_ps[:],
                    o_sb[:, i * 32 : (i + 1) * 32],
                    et[:, i * 256 : (i + 1) * 256],
                    start=(i == 0),
                    stop=(i == 7),
                )

            # fold 256 -> 64 and add rel (rel is zeros on cores 1..7)
            p2_sb = spool.tile([32, 256], f32, tag="p2sb")
            nc.vector.tensor_copy(p2_sb[:], p2_ps[:])
            v1 = spool.tile([32, 64], f32, tag="v1")
            nc.vector.tensor_add(v1[:], p2_sb[:, 0:64], p2_sb[:, 64:128])
            v2 = spool.tile([32, 64], f32, tag="v2")
            nc.vector.tensor_add(v2[:], p2_sb[:, 128:192], p2_sb[:, 192:256])
            nc.vector.tensor_add(v1[:], v1[:], v2[:])
            vs = spool.tile([32, 64], f32, tag="vs")
            nc.vector.tensor_add(vs[:], v1[:], relt[:])

            # transpose v slices: vt[m, i*32+b] = v[b, i*8+m] (partition base 0)
            vt_ps = pp.tile([8, 256], f32, tag="vt_ps")
            for i in range(8):
                nc.tensor.transpose(
                    vt_ps[:, i * 32 : (i + 1) * 32],
                    vs[:, i * 8 : (i + 1) * 8],
                    idt[:],
                )
            vt_sb = spool.tile([8, 256], f32, tag="vt")
            nc.vector.tensor_copy(vt_sb[:], vt_ps[:])

            # stage 3: out_h[(i4,b),(o,j)] = vt[:, h-cols].T @ wn[m,(o,j)]
            for h in range(2):
                o3 = pp3.tile([128, 512], f32, tag="o3")
                nc.tensor.matmul(
                    o3[:],
                    vt_sb[:, h * 128 : (h + 1) * 128],
                    wnt[:],
                    start=True,
                    stop=True,
                )
                o3_sb = spool.tile([128, 512], f32, tag="o3sb")
                nc.vector.tensor_copy(o3_sb[:], o3[:])
                nc.sync.dma_start(out=OUT[h], in_=o3_sb[:])
    nc.finalize()
    return nc


def _prepack(current_pose, w_current, w_next, E_proj, rel_embedd):
    import ml_dtypes

    cp = np.ascontiguousarray(current_pose, dtype=np.float32)
    wc = np.ascontiguousarray(w_current, dtype=np.float32).reshape(64, 8, 16, 8, 8)
    # A_all[c, p, (q,m), (i,b)]
    cp6 = cp.reshape(B, 64, 8, 16, 8, 8)  # (b, p, c, q, i, m)
    a_all = np.ascontiguousarray(cp6.transpose(2, 1, 3, 5, 4, 0)).reshape(
        8, 64, 128, 256
    )
    # Wblk_all[c, p, (q,m), (q',j)] block-diagonal
    w_all = np.zeros((8, 64, 16, 8, 16, 8), dtype=np.float32)
    wc_t = wc.transpose(1, 0, 2, 3, 4)  # (c, p, q, m, j)
    for q in range(16):
        w_all[:, :, q, :, q, :] = wc_t[:, :, q]
    w_all = w_all.reshape(8, 64, 128, 128)
    aw_all = np.concatenate([a_all, w_all], axis=-1)  # (c, p, 128, 384)
    # -> (c, part, (p, x)) flat columns, bf16
    aw_all = np.ascontiguousarray(
        aw_all.transpose(0, 2, 1, 3), dtype=ml_dtypes.bfloat16
    ).reshape(8, 128, 64 * 384)
    # E4[c, i, (q,j), e] with the 1/64 fold baked in
    e5 = (np.asarray(E_proj, dtype=np.float32) / 64.0).reshape(8, 16, 8, 8, 256)
    e_all = np.ascontiguousarray(e5.transpose(0, 2, 1, 3, 4)).reshape(8, 8, 128, 256)
    e_all = np.ascontiguousarray(
        e_all.transpose(0, 2, 1, 3), dtype=ml_dtypes.bfloat16
    ).reshape(8, 128, 2048)
    # rel tile: only core 0 carries the affine term
    rel_all = np.zeros((8, 32, 64), dtype=np.float32)
    rel_all[0] = np.broadcast_to(
        np.asarray(rel_embedd, dtype=np.float32).reshape(1, 64), (32, 64)
    )
    wn_pack = np.ascontiguousarray(
        np.asarray(w_next, dtype=np.float32).transpose(1, 0, 2).reshape(8, 512)
    )
    ident = np.eye(32, dtype=np.float32)
    in_maps = []
    for c in range(NCORES):
        in_maps.append(
            {
                "aw_pack": aw_all[c],
                "e_pack": e_all[c],
                "rel32": rel_all[c],
                "wn_pack": wn_pack,
                "ident32": ident,
            }
        )
    return in_maps


def kernel(current_pose, w_current, w_next, E_proj, rel_embedd):
    from concourse import bass_utils

    if "nc" not in _STATE:
        _STATE["nc"] = _build_nc()
    nc = _STATE["nc"]
    in_maps = _prepack(current_pose, w_current, w_next, E_proj, rel_embedd)
    trace = os.environ.get("KERNEL_TRACE") == "1"
    res = bass_utils.run_bass_kernel_spmd(
        nc, in_maps, core_ids=list(range(NCORES)), trace=trace
    )
    _STATE["last_result"] = res
    acc = np.zeros((2, 128, 512), dtype=np.float32)
    for c in range(NCORES):
        acc += res.results[c]["out"]
    # [h, (i4, b), (o, j)] -> (b, o, h*4+i4, j)
    out = (
        acc.reshape(2, 4, 32, 64, 8)
        .transpose(2, 3, 0, 1, 4)
        .reshape(B, OUT_N, POSE)
    )
    return np.ascontiguousarray(out[:, None, :, :])



# revision 3
# speedup vs baseline: 1.3730x; 1.3730x over previous
"""Trainium2 Bass kernel for nn_LinformerProjectionEntireOutImg.

Math: the reference's softmax is over a constant tensor -> uniform 1/64, so
the whole net collapses to a linear pipeline:
  T[b,q,i,j]  = sum_p cp[b, p*128+q, i, :] @ wc[p*128+q, :, j]   (q = n mod 128)
  S[b, r]     = T.reshape(B, 8192),  r = q*64 + i*8 + j
  P2[b,e]     = S @ E_proj.reshape(8192, 256)
  v[b,k]      = (P2[b,k]+P2[b,64+k]+P2[b,128+k]+P2[b,192+k])/64 + rel[k]
  out[b,o,i,j]= sum_m v[b, i*8+m] * w_next[o, m, j]
Sharding: core c owns capsule groups q in [16c, 16c+16) (== heads 4c..4c+4),
batch unsharded. Each core reads a disjoint 1/8 of current_pose/w_current and
1/8 of E_proj. The pipeline is linear, so each core emits its partial output
(core 0 carries the rel_embedd affine term) and the unshard is a sum.

Precision plan (HBM traffic is the bottleneck; 358 GB/s/core):
  stage 1 operands in fp8e4 (A raw randn; W pre-scaled x64 on host so its
  0.02*randn values sit in e4m3's normal range), fp32 PSUM accumulation.
  The 4-way hid fold (256->64) plus the 1/64 softmax mean plus the 1/64
  W-scale compensation are all folded into E on the host -> E shrinks to
  [128,512] bf16.  Stage 2/3 run in bf16, output in bf16 (host sums cores
  in fp32).  Measured end-to-end rel err ~3e-3 vs the 2e-2 gate.
"""

import os

import numpy as np

_STATE: dict = {}

B, OUT_N, POSE = 32, 64, 64
NCORES = 8

# p-chunk boundaries for the streamed stage-1 operand: small first chunks so
# the first matmul starts early, then large chunks to amortize DMA issue.
P_BOUNDS = [0, 2, 4, 8, 14, 20, 26, 32, 38, 44, 51, 58, 64]
WARMUP_MM = 6  # dummy matmuls to start the PE DVFS ramp during the DMA lead-in


def _build_nc():
    import concourse.mybir as mybir
    from concourse import bacc
    from concourse.tile import TileContext

    f32 = mybir.dt.float32
    bf16 = mybir.dt.bfloat16
    f8 = mybir.dt.float8e4
    nc = bacc.Bacc()
    # AW pack: per p, 256 fp8 cols of A ((i,b) major) then 128 fp8 cols of
    # block-diag W -> 384 B/partition/p.
    AW = nc.dram_tensor("aw_pack", [128, 64 * 384], f8, kind="ExternalInput")
    E = nc.dram_tensor("e_pack", [128, 512], bf16, kind="ExternalInput")
    REL = nc.dram_tensor("rel32", [32, 64], f32, kind="ExternalInput")
    WN = nc.dram_tensor("wn_pack", [8, 512], bf16, kind="ExternalInput")
    IDT = nc.dram_tensor("ident32", [32, 32], bf16, kind="ExternalInput")
    OUT = nc.dram_tensor("out", [2, 128, 512], bf16, kind="ExternalOutput")

    with TileContext(nc) as tc:
        with (
            tc.tile_pool(name="apool", bufs=len(P_BOUNDS) - 1) as apool,
            tc.tile_pool(name="cpool", bufs=1) as cpool,
            tc.tile_pool(name="spool", bufs=1) as spool,
            tc.tile_pool(name="pp", bufs=1, space="PSUM") as pp,
            tc.tile_pool(name="pp3", bufs=2, space="PSUM") as pp3,
        ):
            # PE warmup: dummy matmuls on a zeroed scratch tile keep the PE
            # busy during the DMA lead-in so the DVFS ramp starts early.
            zt = cpool.tile([128, 256], f8, tag="zt")
            nc.gpsimd.memset(zt[:], 0)
            warm_ps = pp.tile([128, 256], f32, tag="warm")
            for _ in range(WARMUP_MM):
                nc.tensor.matmul(
                    warm_ps[:], zt[:, 0:128], zt[:], start=True, stop=True
                )

            # AW chunk DMAs, alternating between the two HWDGE queues.
            awts = []
            for ci in range(len(P_BOUNDS) - 1):
                w = (P_BOUNDS[ci + 1] - P_BOUNDS[ci]) * 384
                awt = apool.tile([128, w], f8, tag="aw")
                eng = (nc.sync, nc.scalar)[ci % 2]
                eng.dma_start(
                    out=awt[:],
                    in_=AW[:, P_BOUNDS[ci] * 384 : P_BOUNDS[ci + 1] * 384],
                )
                awts.append(awt)
            # params land after the A stream: they're only needed at stage 2/3
            et = cpool.tile([128, 512], bf16, tag="e")
            nc.scalar.dma_start(out=et[:], in_=E[:])
            relt = cpool.tile([32, 64], f32, tag="rel")
            nc.sync.dma_start(out=relt[:], in_=REL[:])
            idt = cpool.tile([32, 32], bf16, tag="idt")
            nc.scalar.dma_start(out=idt[:], in_=IDT[:])
            wnt = cpool.tile([8, 512], bf16, tag="wn")
            nc.sync.dma_start(out=wnt[:], in_=WN[:])

            # stage 1: T[(q,j),(i,b)] = sum_p Wblk_p.T @ A_p  (block-diag over q)
            # Two interleaved accumulation chains (even/odd p) in separate
            # PSUM banks so per-matmul ordering waits don't serialize the PE.
            o_ps0 = pp.tile([128, 256], f32, tag="o_ps0")
            o_ps1 = pp.tile([128, 256], f32, tag="o_ps1")
            for ci in range(len(P_BOUNDS) - 1):
                awt = awts[ci]
                for t in range(P_BOUNDS[ci + 1] - P_BOUNDS[ci]):
                    p = P_BOUNDS[ci] + t
                    tgt = o_ps0 if p % 2 == 0 else o_ps1
                    nc.tensor.matmul(
                        tgt[:],
                        awt[:, t * 384 + 256 : (t + 1) * 384],
                        awt[:, t * 384 : t * 384 + 256],
                        start=(p < 2),
                        stop=(p >= 62),
                    )
            o_half = spool.tile([128, 256], f32, tag="ohalf")
            nc.vector.tensor_copy(o_half[:], o_ps0[:])
            o_sb = spool.tile([128, 256], bf16, tag="osb")
            nc.vector.tensor_add(o_sb[:], o_half[:], o_ps1[:])

            # stage 2: v[b,k] += O[:, i-cols].T @ Ef_i  (accumulate over i)
            # Ef has the 4-way hid fold, the softmax 1/64 and the W x64
            # compensation baked in.
            p2_ps = pp.tile([32, 64], f32, tag="p2_ps")
            for i in range(8):
                nc.tensor.matmul(
                    p2_ps[:],
                    o_sb[:, i * 32 : (i + 1) * 32],
                    et[:, i * 64 : (i + 1) * 64],
                    start=(i == 0),
                    stop=(i == 7),
                )

            # add rel (zeros on cores 1..7), cast to bf16
            vs = spool.tile([32, 64], bf16, tag="vs")
            nc.vector.tensor_add(vs[:], p2_ps[:], relt[:])

            # transpose v slices: vt[m, i*32+b] = v[b, i*8+m] (partition base 0)
            vt_ps = pp.tile([8, 256], bf16, tag="vt_ps")
            for i in range(8):
                nc.tensor.transpose(
                    vt_ps[:, i * 32 : (i + 1) * 32],
                    vs[:, i * 8 : (i + 1) * 8],
                    idt[:],
                )
            vt_sb = spool.tile([8, 256], bf16, tag="vt")
            nc.vector.tensor_copy(vt_sb[:], vt_ps[:])

            # stage 3: out_h[(i4,b),(o,j)] = vt[:, h-cols].T @ wn[m,(o,j)]
            for h in range(2):
                o3 = pp3.tile([128, 512], f32, tag="o3")
                nc.tensor.matmul(
                    o3[:],
                    vt_sb[:, h * 128 : (h + 1) * 128],
                    wnt[:],
                    start=True,
                    stop=True,
                )
                o3_sb = spool.tile([128, 512], bf16, tag="o3sb")
                if h == 0:
                    nc.vector.tensor_copy(o3_sb[:], o3[:])
                else:
                    nc.scalar.copy(o3_sb[:], o3[:])
                (nc.sync, nc.scalar)[h].dma_start(out=OUT[h], in_=o3_sb[:])
    nc.finalize()
    return nc


def _prepack(current_pose, w_current, w_next, E_proj, rel_embedd):
    import ml_dtypes

    f8 = ml_dtypes.float8_e4m3fn
    cp = np.ascontiguousarray(current_pose, dtype=np.float32)
    wc = np.ascontiguousarray(w_current, dtype=np.float32).reshape(64, 8, 16, 8, 8)
    # A_all[c, p, (q,m), (i,b)]
    cp6 = cp.reshape(B, 64, 8, 16, 8, 8)  # (b, p, c, q, i, m)
    a_all = np.ascontiguousarray(cp6.transpose(2, 1, 3, 5, 4, 0), dtype=f8).reshape(
        8, 64, 128, 256
    )
    # Wblk_all[c, p, (q,m), (q',j)] block-diagonal; x64 so 0.02*randn values
    # land in e4m3's normal range (compensated in E)
    w_all = np.zeros((8, 64, 16, 8, 16, 8), dtype=f8)
    wc_t = np.ascontiguousarray(wc.transpose(1, 0, 2, 3, 4)) * 64.0  # (c,p,q,m,j)
    wc_t8 = wc_t.astype(f8)
    for q in range(16):
        w_all[:, :, q, :, q, :] = wc_t8[:, :, q]
    w_all = w_all.reshape(8, 64, 128, 128)
    aw_all = np.concatenate([a_all, w_all], axis=-1)  # (c, p, 128, 384)
    # -> (c, part, (p, x)) flat columns
    aw_all = np.ascontiguousarray(aw_all.transpose(0, 2, 1, 3)).reshape(
        8, 128, 64 * 384
    )
    # Ef[c, (q,j), (i,k)]: hid fold (sum over h4), /64 softmax mean, /64 W-scale
    e6 = (np.asarray(E_proj, dtype=np.float32) / 4096.0).reshape(
        8, 4, 4, 8, 8, 4, 64
    )  # (c, qh, q4, i, j, h4, k)
    ef = e6.sum(axis=5)  # (c, qh, q4, i, j, k)
    ef = np.ascontiguousarray(
        ef.transpose(0, 1, 2, 4, 3, 5), dtype=ml_dtypes.bfloat16
    ).reshape(8, 128, 512)  # (c, (qh,q4,j), (i,k))
    # rel tile: only core 0 carries the affine term
    rel_all = np.zeros((8, 32, 64), dtype=np.float32)
    rel_all[0] = np.broadcast_to(
        np.asarray(rel_embedd, dtype=np.float32).reshape(1, 64), (32, 64)
    )
    wn_pack = np.ascontiguousarray(
        np.asarray(w_next, dtype=np.float32).transpose(1, 0, 2).reshape(8, 512),
        dtype=ml_dtypes.bfloat16,
    )
    ident = np.eye(32, dtype=ml_dtypes.bfloat16)
    in_maps = []
    for c in range(NCORES):
        in_maps.append(
            {
                "aw_pack": aw_all[c],
                "e_pack": ef[c],
                "rel32": rel_all[c],
                "wn_pack": wn_pack,
                "ident32": ident,
            }
        )
    return in_maps


def kernel(current_pose, w_current, w_next, E_proj, rel_embedd):
    from concourse import bass_utils

    if "nc" not in _STATE:
        _STATE["nc"] = _build_nc()
    nc = _STATE["nc"]
    in_maps = _prepack(current_pose, w_current, w_next, E_proj, rel_embedd)
    trace = os.environ.get("KERNEL_TRACE") == "1"
    res = bass_utils.run_bass_kernel_spmd(
        nc, in_maps, core_ids=list(range(NCORES)), trace=trace
    )
    _STATE["last_result"] = res
    acc = np.zeros((2, 128, 512), dtype=np.float32)
    for c in range(NCORES):
        acc += np.asarray(res.results[c]["out"], dtype=np.float32)
    # [h, (i4, b), (o, j)] -> (b, o, h*4+i4, j)
    out = (
        acc.reshape(2, 4, 32, 64, 8)
        .transpose(2, 3, 0, 1, 4)
        .reshape(B, OUT_N, POSE)
    )
    return np.ascontiguousarray(out[:, None, :, :])
